# revision 36
# baseline (speedup 1.0000x reference)
# Binary linear: y[b,s,o] = sum_i x[b,s,i] * sign(W)[o,i]
#
# Strategy (8 NeuronCores, data-parallel over tokens):
#   - Host: flatten x to [32768, 768], shard 8 x [4096, 768], transpose each
#     shard to xT [768, 4096] bf16, tiled per token chunk as
#     [128p, 6ksub, cw] so each chunk is one contiguous-row DMA. The
#     binarized weight sign(W).T is replicated per core as fp8e4 (+-1 is
#     exact in fp8; mixed fp8-stationary x bf16-moving matmuls run at full
#     rate and halve the weight DMA), so only x carries bf16 rounding.
#   - Device (per core): weight-stationary matmuls:
#     loop token chunk -> out block (128) -> 6 matmuls accumulating
#     psum[128o, cw] += w[128i,128o].T @ x[128i,cw] over the 6 k-subtiles.
#     Wide chunks keep every matmul at the PE stream floor (213ns for
#     N=512); edge chunks (384/256) shorten the pipeline fill and drain.
#     A 22-matmul PE warmup during the framework preamble ramps the
#     p-state/HAM clock so the body runs at 2.4GHz from its first matmul.
#     All first-use data (w0, x0, w1..w5, x1..) streams on the single sync
#     DMA queue in exact consumption order — deterministic, no cross-queue
#     HBM bandwidth races. psum -> SBUF copies cast to bf16 (halves store
#     traffic) alternating DVE/ACT; stores ride the scalar queue; the last
#     chunk stores per out-block with the final store split across both
#     HWDGE queues for a minimal drain.
#   - Host: reassemble [4, 8192, 768] f32 from the bf16 tile layout.
#
# Measured on 8x trn2 NeuronCores: ~80.0us HW exec (traced), rel err 2.3e-3.
# Body is at the bf16 PE stream floor (62.5us for 147456 rows @2.4GHz);
# fp8 DoubleRow was measured to stream at the same rate per instruction
# (2x FLOPs via K=256), so an fp8 hi+lo split of x would tie, not win.

import numpy as np

N_CORES = 8
B, S, D = 4, 8192, 768
T_TOTAL = B * S
T_CORE = T_TOTAL // N_CORES   # 4096 tokens per core
P = 128
NS = 6                        # k-subtiles of 128
OB = D // P                   # 6 out blocks
CH = [384] + [512] * 6 + [384, 256]
assert sum(CH) == T_CORE
NCH = len(CH)
CW = 512                      # max chunk width (row padding in DRAM)

_cache = {}


def _build():
    import concourse.bacc as bacc
    import concourse.mybir as mybir
    import concourse.tile as tile

    f32 = mybir.dt.float32
    bf16 = mybir.dt.bfloat16
    fp8 = mybir.dt.float8e4

    nc = bacc.Bacc(
        "TRN2",
        target_bir_lowering=False,
        debug=False,
        num_devices=N_CORES,
    )

    xD = nc.dram_tensor("x8", [NCH * P, NS * CW], bf16, kind="ExternalInput")
    wD = nc.dram_tensor("w8", [OB * P, NS * P], fp8, kind="ExternalInput")
    yD = nc.dram_tensor("y8", [NCH * 2 * P, 3 * CW], bf16, kind="ExternalOutput")

    with tile.TileContext(nc) as tc_:
        with (
            tc_.tile_pool(name="wbuf", bufs=1) as wpool,
            tc_.tile_pool(name="xbuf", bufs=1) as xpool,
            tc_.tile_pool(name="ybuf", bufs=4) as ypool,
            tc_.tile_pool(name="psum", bufs=7, space="PSUM") as psum_pool,
        ):
            # --- PE warmup: dummy matmuls (on whatever SBUF holds; outputs
            # never read) during the framework preamble / first DMAs so the
            # p-state + HAM clock gate are at full rate when the real
            # matmuls start. N=256 keeps each one short so warmup ends
            # about when the first x chunk lands. ---
            wu = xpool.tile([P, 256], bf16, tag="warmup", name="wu")
            nc.gpsimd.memset(wu[:], 0.0)
            wups = psum_pool.tile([P, 512], f32, tag="wups", name="wups", bufs=1)
            for k in range(22):
                nc.tensor.matmul(
                    wups[:, :256], wu[:, :P], wu[:, :256],
                    start=True, stop=True, skip_group_check=True,
                )

            # All first-use data goes on the single sync queue in exact
            # consumption order (w0, x0, w1..w5, x1..): one FIFO stream is
            # deterministic — two concurrent queues share HBM unpredictably
            # and the loser's weights arrive late, stalling the early body.
            # The x tiles share one tag with bufs=3 so x3.. only load once
            # their ring slot's previous chunk is consumed.
            wtiles = [
                wpool.tile([P, NS * P], fp8, tag=f"w{ob}", name=f"w{ob}")
                for ob in range(OB)
            ]
            xtiles = [
                xpool.tile([P, NS * CW], bf16, tag="x", name=f"x{c}", bufs=3)
                for c in range(NCH)
            ]

            def w_load(ob):
                nc.sync.dma_start(wtiles[ob][:], wD[ob * P:(ob + 1) * P, :])

            def x_load(c):
                nc.sync.dma_start(
                    xtiles[c][:, :NS * CH[c]], xD[c * P:(c + 1) * P, :NS * CH[c]]
                )

            w_load(0)
            x_load(0)
            for ob in range(1, OB):
                w_load(ob)
            for c in range(1, NCH):
                x_load(c)

            wt = [t.rearrange("p (i o) -> p i o", i=NS) for t in wtiles]
            xt = [
                xtiles[c][:, :NS * CH[c]].rearrange("p (s t) -> p s t", s=NS)
                for c in range(NCH)
            ]

            for c in range(NCH - 1):
                cw = CH[c]
                for half in range(2):
                    yt = ypool.tile([P, 3 * cw], bf16, tag="y3", name=f"y{c}_{half}")
                    y3 = yt.rearrange("p (g t) -> p g t", g=3)
                    yrow = (c * 2 + half) * P
                    for g in range(3):
                        ob = half * 3 + g
                        ps = psum_pool.tile([P, 512], f32, tag="ps", name=f"ps{c}_{ob}")
                        for i in range(NS):
                            nc.tensor.matmul(
                                ps[:, :cw],
                                wt[ob][:, i, :],
                                xt[c][:, i, :],
                                start=(i == 0),
                                stop=(i == NS - 1),
                            )
                        if ob % 2 == 0:
                            nc.vector.tensor_copy(y3[:, g, :], ps[:, :cw])
                        else:
                            nc.scalar.copy(y3[:, g, :], ps[:, :cw])
                    nc.scalar.dma_start(yD[yrow:yrow + P, :3 * cw], yt[:])

            # --- last chunk: per-ob tiles, stores issued as soon as each
            # copy lands (alternating sync/scalar queues). The final group's
            # copy goes to the DVE and its store is split across both queues
            # so the drain after the last matmul is minimal. ---
            c = NCH - 1
            cw = CH[c]
            for idx, ob in enumerate((1, 3, 5, 0, 2, 4)):
                fin = idx == 5
                half, g = divmod(ob, 3)
                yrow = (c * 2 + half) * P
                yl = ypool.tile([P, cw], bf16, tag="yl", name=f"yl{ob}", bufs=6)
                ps = psum_pool.tile([P, 512], f32, tag="ps", name=f"ps{c}_{ob}")
                for i in range(NS):
                    nc.tensor.matmul(
                        ps[:, :cw],
                        wt[ob][:, i, :],
                        xt[c][:, i, :],
                        start=(i == 0),
                        stop=(i == NS - 1),
                    )
                if idx % 2 == 0 and not fin:
                    nc.scalar.copy(yl[:], ps[:, :cw])
                else:
                    nc.vector.tensor_copy(yl[:], ps[:, :cw])
                dst = yD[yrow:yrow + P, g * cw:(g + 1) * cw]
                if fin:
                    # split by partition rows (keeps 512B descriptors; a
                    # column split would halve them under the 512B knee).
                    # Both HWDGE queues are clear here: the penultimate
                    # store went to gpsimd, earlier ones have drained.
                    nc.sync.dma_start(dst[:P // 2, :], yl[:P // 2, :])
                    nc.scalar.dma_start(dst[P // 2:, :], yl[P // 2:, :])
                else:
                    eng = (nc.scalar, nc.sync, nc.scalar, nc.sync, nc.gpsimd)[idx]
                    eng.dma_start(dst, yl[:])

    nc.compile()
    return nc


def _get_nc():
    if "nc" not in _cache:
        _cache["nc"] = _build()
    return _cache["nc"]


def _prep_inputs(x, weight):
    import ml_dtypes

    bf = ml_dtypes.bfloat16
    x = np.asarray(x, dtype=np.float32).reshape(N_CORES, T_CORE, D)
    w = np.asarray(weight, dtype=np.float32)

    # w8[ob*P+p, i*P+o] = sign(W).T[i*128+p, ob*128+o]; +-1 exact in bf16
    sT = np.sign(w).T
    w8 = np.ascontiguousarray(
        sT.reshape(NS, P, OB, P).transpose(2, 1, 0, 3)
    ).reshape(OB * P, NS * P).astype(ml_dtypes.float8_e4m3)

    in_maps = []
    for c in range(N_CORES):
        xc = np.ascontiguousarray(x[c].T).astype(bf)    # [768, 4096] bf16
        x3 = xc.reshape(NS, P, T_CORE)                  # (s, p, tok)
        x8 = np.zeros((NCH * P, NS * CW), dtype=bf)
        t0 = 0
        for ci, cw in enumerate(CH):
            blk = x3[:, :, t0:t0 + cw].transpose(1, 0, 2)   # (p, s, t)
            x8[ci * P:(ci + 1) * P, :NS * cw] = blk.reshape(P, NS * cw)
            t0 += cw
        in_maps.append({"x8": x8, "w8": w8})
    return in_maps


def _decode_out(res):
    # y8 rows [(c, h, p)], cols [(g, t)] -> y[tok, o]:
    #   tok = start_c + t, o = (3h+g)*128 + p
    outs = []
    for c in range(N_CORES):
        y8 = np.asarray(res.results[c]["y8"])
        yc = np.empty((T_CORE, D), dtype=np.float32)
        t0 = 0
        for ci, cw in enumerate(CH):
            for h in range(2):
                blk = y8[(ci * 2 + h) * P:(ci * 2 + h + 1) * P, :3 * cw]
                blk = blk.reshape(P, 3, cw).astype(np.float32)
                # o = (3h+g)*128+p, tok = t0+t
                yc[t0:t0 + cw, h * 384:(h + 1) * 384] = (
                    blk.transpose(2, 1, 0).reshape(cw, 384)
                )
            t0 += cw
        outs.append(yc)
    y = np.concatenate(outs, axis=0)
    return y.reshape(B, S, D)


def _install_axon_ntff_hook():
    """The agent image's `antenv` lacks `axon_hooks`; register an equivalent
    module backed by direct ctypes calls into libaxon_pjrt.so so that
    run_bass_kernel_spmd(trace=True) can capture NTFF profiles under axon."""
    import sys

    if "antenv.axon_hooks" in sys.modules:
        return
    import contextlib
    import ctypes
    import types

    so_path = "/opt/axon/libaxon_pjrt.so"
    try:
        lib = ctypes.CDLL(so_path)
    except OSError:
        return
    if not hasattr(lib, "axon_start_nrt_profile"):
        return
    lib.axon_start_nrt_profile.argtypes = [
        ctypes.POINTER(ctypes.c_int64),
        ctypes.c_size_t,
    ]
    lib.axon_start_nrt_profile.restype = ctypes.c_int64
    lib.axon_stop_nrt_profile.argtypes = [ctypes.c_char_p]
    lib.axon_stop_nrt_profile.restype = ctypes.c_int64

    @contextlib.contextmanager
    def _hook(output_dir, device_ids):
        import jax

        jax.devices()
        if device_ids:
            ids = (ctypes.c_int64 * len(device_ids))(*device_ids)
            rc = lib.axon_start_nrt_profile(ids, len(device_ids))
        else:
            rc = lib.axon_start_nrt_profile(None, 0)
        if rc != 0:
            raise RuntimeError(f"axon_start_nrt_profile rc={rc}")
        try:
            yield
        finally:
            n = lib.axon_stop_nrt_profile(str(output_dir).encode())
            print(f"ntff profile: {n} file(s) written to {output_dir}")

    mod = types.ModuleType("antenv.axon_hooks")
    mod.get_axon_ntff_profile_hook = lambda: _hook
    mod.set_axon_ntff_profile_hook = lambda h: None
    sys.modules["antenv.axon_hooks"] = mod


def _run(x, weight, trace=False):
    from concourse.bass_utils import run_bass_kernel_spmd

    if trace:
        _install_axon_ntff_hook()
    nc = _get_nc()
    in_maps = _prep_inputs(x, weight)
    res = run_bass_kernel_spmd(
        nc, in_maps, core_ids=list(range(N_CORES)), trace=trace
    )
    return _decode_out(res), res


def kernel(x, weight):
    out, _ = _run(x, weight, trace=False)
    return out
